# revision 39
# baseline (speedup 1.0000x reference)
"""Aleatoric cross-entropy loss on 8 Trainium2 NeuronCores.

Math (per reference):
  ce(logits) = mean_b( sum_c( -log_softmax(logits)_bc * true_bc ) / 10 )
  std   = sqrt(logit_var)                                  [B,1]
  z_tbc = pred_bc + std_b * noise_tbc                      [T,B,C]
  distorted_t = ce(z_t)
  outputs: (mean_t distorted, mean_t -elu(undist - distorted_t),
            undist = ce(pred), mean_b exp(logit_var) - 1)

Per-row (t,b) decomposition (no max-subtraction needed: |z| <= ~10):
  lse_tb = log(sum_c exp(z)) = log(sum_c exp(pred)_bc * exp(std_b*noise)_tbc)
  dot_tb = sum_c z*true = pdot_b + sum_c (std_b*noise_tbc)*true_bc
  ce_tb  = (lse_tb * strue_b - dot_tb) / 10

Device (per core, 64 batch rows = data-parallel over B): everything is cast
to a pair of TensorE contractions over c. The host ships
  ns_T [C', R] = (std * noise)^T   (bf16, c-padded to 3072, R = 25*64 rows)
  ew   [c-chunk panels of exp(pred)^T]   tw [panels of true^T]
and per 128-row c-chunk the device runs: DMA chunk -> 4 matmuls
(true-weights x raw chunk, accumulating gram_n [64, R] in PSUM) -> ACT exp
(the only elementwise pass) -> 4 matmuls (exp(pred)-weights x exp-chunk,
accumulating gram_s). The host reads the [64, R] grams' generalized
diagonals (row b of column (t,b)), which are exactly sum_c exp(z) and
sum_c (std*noise)*true per (t,b), and finishes the tiny [T]-vector math in
float64. VectorE does nothing but the final PSUM->SBUF drains.
"""

import ml_dtypes
import numpy as np

B, C, T = 512, 3000, 25
NCORES = 8
BL = B // NCORES            # 64 batch rows per core
R = T * BL                  # 1600 (t,b) columns per core
P = 128                     # contraction rows per chunk
CP = 3072                   # C padded to a multiple of 128
KCH = CP // P               # 24 c-chunks
NSL = 4                     # 512-wide matmul slices per [64, R] gram
SLICES = [(0, 512), (512, 1024), (1024, 1536), (1536, R)]

_PROGRAM = None  # cached Bass program
_LAST = [None]   # last BassKernelResults (for test harness introspection)


def _build_program():
    import concourse.bass as bass
    import concourse.tile as tile
    from concourse import bacc, mybir

    f32 = mybir.dt.float32
    bf16 = mybir.dt.bfloat16
    Act = mybir.ActivationFunctionType

    nc = bacc.Bacc("TRN2", target_bir_lowering=False, debug=False,
                   num_devices=NCORES)

    noise_d = nc.dram_tensor("noiset", [CP, R], bf16,
                             kind="ExternalInput").ap()
    ew_d = nc.dram_tensor("ew", [P, KCH * BL], bf16,
                          kind="ExternalInput").ap()
    tw_d = nc.dram_tensor("tw", [P, KCH * BL], bf16,
                          kind="ExternalInput").ap()

    gs_d = nc.dram_tensor("gs", [BL, R], f32, kind="ExternalOutput").ap()
    gn_d = nc.dram_tensor("gn", [BL, R], f32, kind="ExternalOutput").ap()

    with tile.TileContext(nc) as tc:
        with (
            tc.tile_pool(name="const", bufs=1) as const_pool,
            tc.tile_pool(name="noise", bufs=5) as noise_pool,
            tc.tile_pool(name="fexp", bufs=3) as f_pool,
            tc.tile_pool(name="out", bufs=1) as out_pool,
            tc.tile_pool(name="psum", bufs=1, space="PSUM") as psum_pool,
        ):
            ew = const_pool.tile([P, KCH * BL], bf16)
            tw = const_pool.tile([P, KCH * BL], bf16)

            # Load the exp table set first thing via a dependency-free dummy
            # activation on a const AP, overlapping the first DMAs.
            actwarm = const_pool.tile([1, 1], f32)
            nc.scalar.activation(out=actwarm[:, :],
                                 in_=nc.const_aps.tensor(0.0, (1, 1)),
                                 func=Act.Exp)

            ps_s = psum_pool.tile([BL, R], f32)
            ps_n = psum_pool.tile([BL, R], f32)

            # groups: first chunks ride alone so the pipeline ramps early,
            # then pairs amortize the ACT per-op overhead (one FD=3200 exp)
            groups = [[0], [1]] + [[k, k + 1] for k in range(2, KCH, 2)]
            for grp in groups:
                g = len(grp)
                nk = noise_pool.tile([P, 2 * R], bf16, tag="nk")
                if g == 1:
                    nc.sync.dma_start(out=nk[:, 0:R],
                                      in_=noise_d[grp[0] * P:
                                                  (grp[0] + 1) * P, :])
                else:
                    # one DMA for all chunks of the group
                    nc.sync.dma_start(
                        out=nk[:, 0:g * R].rearrange("p (j q) -> p j q", j=g),
                        in_=noise_d[grp[0] * P:(grp[0] + g) * P, :].rearrange(
                            "(j p) q -> p j q", j=g))
                if grp == [0]:
                    nc.sync.dma_start(out=tw[:, :], in_=tw_d[:, :])
                elif grp == [1]:
                    nc.sync.dma_start(out=ew[:, :], in_=ew_d[:, :])
                for j, k in enumerate(grp):
                    for q0, q1 in SLICES:
                        nc.tensor.matmul(ps_n[:, q0:q1],
                                         tw[:, k * BL:(k + 1) * BL],
                                         nk[:, j * R + q0:j * R + q1],
                                         start=(k == 0), stop=(k == KCH - 1))
                fk = f_pool.tile([P, 2 * R], bf16, tag="fk")
                nc.scalar.activation(out=fk[:, 0:g * R], in_=nk[:, 0:g * R],
                                     func=Act.Exp)
                for j, k in enumerate(grp):
                    for q0, q1 in SLICES:
                        nc.tensor.matmul(ps_s[:, q0:q1],
                                         ew[:, k * BL:(k + 1) * BL],
                                         fk[:, j * R + q0:j * R + q1],
                                         start=(k == 0), stop=(k == KCH - 1))

            gs_sb = out_pool.tile([BL, R], f32)
            gn_sb = out_pool.tile([BL, R], f32)
            nc.vector.tensor_copy(gn_sb[:, :], ps_n[:, :])
            nc.sync.dma_start(out=gn_d[:, :], in_=gn_sb[:, :])
            # split the gram_s drain across the two idle engines
            nc.scalar.copy(gs_sb[:, 0:800], ps_s[:, 0:800])
            nc.vector.tensor_copy(gs_sb[:, 800:R], ps_s[:, 800:R])
            nc.sync.dma_start(out=gs_d[:, :], in_=gs_sb[:, :])

    nc.compile()
    return nc


def _get_program():
    global _PROGRAM
    if _PROGRAM is None:
        _PROGRAM = _build_program()
    return _PROGRAM


def _run_on_device(in_maps, trace=False):
    from concourse.bass_utils import run_bass_kernel_spmd
    nc = _get_program()
    return run_bass_kernel_spmd(nc, in_maps, list(range(NCORES)), trace=trace)


def _weight_panels(mat_cb):
    """[C, BL] -> [P, KCH*BL] bf16: panel k holds rows 128k:128(k+1)."""
    out = np.zeros((CP, BL), np.float32)
    out[:C] = mat_cb
    return np.ascontiguousarray(
        out.reshape(KCH, P, BL).transpose(1, 0, 2).reshape(P, KCH * BL)
    ).astype(ml_dtypes.bfloat16)


def kernel(logit_var, pred, true, noise, _trace=False):
    logit_var = np.asarray(logit_var, dtype=np.float32)
    pred = np.asarray(pred, dtype=np.float32)
    true = np.asarray(true, dtype=np.float32)
    noise = np.asarray(noise, dtype=np.float32)

    std = np.sqrt(logit_var)                     # [B,1] f32
    bf = ml_dtypes.bfloat16

    in_maps = []
    for i in range(NCORES):
        b0 = i * BL
        bsl = slice(b0, b0 + BL)
        ns = noise[:, bsl, :] * std[None, bsl, :]     # [T,BL,C] f32
        nsT = np.zeros((CP, R), np.float32)
        nsT[:C] = ns.reshape(R, C).T
        in_maps.append({
            "noiset": nsT.astype(bf),
            "ew": _weight_panels(np.exp(pred[bsl]).T),
            "tw": _weight_panels(true[bsl].T),
        })

    out = _run_on_device(in_maps, trace=_trace)
    _LAST[0] = out
    results = out.results

    # Gather: gram column q = t*BL + b_local pairs with weight row b_local.
    sumexp = np.empty((T, B), np.float64)
    ndots = np.empty((T, B), np.float64)            # sum_c (std*noise)*true
    bidx = np.arange(R) % BL
    qidx = np.arange(R)
    for i in range(NCORES):
        r = results[i]
        gs = np.asarray(r["gs"], np.float64)        # [BL, R]
        gn = np.asarray(r["gn"], np.float64)
        b0 = i * BL
        sumexp[:, b0:b0 + BL] = gs[bidx, qidx].reshape(T, BL)
        ndots[:, b0:b0 + BL] = gn[bidx, qidx].reshape(T, BL)

    true64 = true.astype(np.float64)
    pred64 = pred.astype(np.float64)
    strue = true64.sum(axis=1)                      # [B]
    pdot = (pred64 * true64).sum(axis=1)            # [B]
    sexp_pred = np.exp(pred64).sum(axis=1)          # [B]

    lse = np.log(sumexp)                            # [T,B]
    dot = pdot[None, :] + ndots                     # [T,B]
    ce = (lse * strue[None, :] - dot) / 10.0        # [T,B]
    distorted = ce.mean(axis=1)                     # [T]
    undist = ((np.log(sexp_pred) * strue - pdot) / 10.0).mean()
    gce = distorted.mean()
    x = undist - distorted
    vloss = np.mean(-np.where(x > 0, x, np.expm1(x)))
    vd = np.mean(np.exp(logit_var[:, 0].astype(np.float64))) - 1.0

    return (np.float32(gce), np.float32(vloss), np.float32(undist),
            np.float32(vd))


# revision 41
# speedup vs baseline: 1.1542x; 1.1542x over previous
"""Aleatoric cross-entropy loss on 8 Trainium2 NeuronCores.

Math (per reference):
  ce(logits) = mean_b( sum_c( -log_softmax(logits)_bc * true_bc ) / 10 )
  std   = sqrt(logit_var)                                  [B,1]
  z_tbc = pred_bc + std_b * noise_tbc                      [T,B,C]
  distorted_t = ce(z_t)
  outputs: (mean_t distorted, mean_t -elu(undist - distorted_t),
            undist = ce(pred), mean_b exp(logit_var) - 1)

Per-row (t,b) decomposition (no max-subtraction needed: |z| <= ~10):
  lse_tb = log(sum_c exp(z)) = log(sum_c exp(pred)_bc * exp(std_b*noise)_tbc)
  dot_tb = sum_c z*true = pdot_b + sum_c (std_b*noise_tbc)*true_bc
  ce_tb  = (lse_tb * strue_b - dot_tb) / 10

Device (per core, 64 batch rows = data-parallel over B): everything is cast
to a pair of TensorE contractions over c. The host ships
  ns_T [C', R] = (std * noise)^T   (bf16, c-padded to 3072, R = 25*64 rows)
  ew   [c-chunk panels of exp(pred)^T]   tw [panels of true^T]
and per 128-row c-chunk (chunks processed in pairs so one FD=3200 ACT op
covers both) the device runs: DMA chunk -> 4 matmuls (true-weights x raw
chunk, accumulating gram_n [64, R] in PSUM) -> ACT exp (the only
elementwise pass) -> 4 matmuls (exp(pred)-weights x exp-chunk, accumulating
gram_s). The host reads the [64, R] grams' generalized diagonals (row b of
column (t,b)), which are exactly sum_c exp(z) and sum_c (std*noise)*true
per (t,b), and finishes the tiny [T]-vector math in float64. The Gram is
64x more MACs than needed, but TensorE (78 TF/s bf16, otherwise idle) does
it in ~the time ACT needs for the exps, so both reductions are free; DVE
only helps drain PSUM. Engine busy ~ ACT 37us / PE 38us / DMA 38us.
"""

import ml_dtypes
import numpy as np

B, C, T = 512, 3000, 25
NCORES = 8
BL = B // NCORES            # 64 batch rows per core
R = T * BL                  # 1600 (t,b) columns per core
P = 128                     # contraction rows per chunk
CP = 3072                   # C padded to a multiple of 128
KCH = CP // P               # 24 c-chunks
NSL = 4                     # 512-wide matmul slices per [64, R] gram
SLICES = [(0, 512), (512, 1024), (1024, 1536), (1536, R)]

_PROGRAM = None  # cached Bass program
_LAST = [None]   # last BassKernelResults (for test harness introspection)


def _build_program():
    import concourse.bass as bass
    import concourse.tile as tile
    from concourse import bacc, mybir

    f32 = mybir.dt.float32
    bf16 = mybir.dt.bfloat16
    Act = mybir.ActivationFunctionType

    nc = bacc.Bacc("TRN2", target_bir_lowering=False, debug=False,
                   num_devices=NCORES)

    noise_d = nc.dram_tensor("noiset", [CP, R], bf16,
                             kind="ExternalInput").ap()
    ew_d = nc.dram_tensor("ew", [P, KCH * BL], bf16,
                          kind="ExternalInput").ap()
    tw_d = nc.dram_tensor("tw", [P, KCH * BL], bf16,
                          kind="ExternalInput").ap()

    gs_d = nc.dram_tensor("gs", [BL, R], f32, kind="ExternalOutput").ap()
    gn_d = nc.dram_tensor("gn", [BL, R], f32, kind="ExternalOutput").ap()

    with tile.TileContext(nc) as tc:
        with (
            tc.tile_pool(name="const", bufs=1) as const_pool,
            tc.tile_pool(name="noise", bufs=5) as noise_pool,
            tc.tile_pool(name="fexp", bufs=3) as f_pool,
            tc.tile_pool(name="out", bufs=1) as out_pool,
            tc.tile_pool(name="psum", bufs=1, space="PSUM") as psum_pool,
        ):
            ew = const_pool.tile([P, KCH * BL], bf16)
            tw = const_pool.tile([P, KCH * BL], bf16)

            # Load the exp table set first thing via a dependency-free dummy
            # activation on a const AP, overlapping the first DMAs.
            actwarm = const_pool.tile([1, 1], f32)
            nc.scalar.activation(out=actwarm[:, :],
                                 in_=nc.const_aps.tensor(0.0, (1, 1)),
                                 func=Act.Exp)

            ps_s = psum_pool.tile([BL, R], f32)
            ps_n = psum_pool.tile([BL, R], f32)

            # groups: first chunks ride alone so the pipeline ramps early,
            # then pairs amortize the ACT per-op overhead (one FD=3200 exp)
            groups = [[0], [1]] + [[k, k + 1] for k in range(2, KCH, 2)]
            for grp in groups:
                g = len(grp)
                nk = noise_pool.tile([P, 2 * R], bf16, tag="nk")
                if g == 1:
                    nc.sync.dma_start(out=nk[:, 0:R],
                                      in_=noise_d[grp[0] * P:
                                                  (grp[0] + 1) * P, :])
                else:
                    # one DMA for all chunks of the group
                    nc.sync.dma_start(
                        out=nk[:, 0:g * R].rearrange("p (j q) -> p j q", j=g),
                        in_=noise_d[grp[0] * P:(grp[0] + g) * P, :].rearrange(
                            "(j p) q -> p j q", j=g))
                if grp == [0]:
                    # weights ride behind the first noise chunk; both must be
                    # emitted before any matmul that reads them (trace order
                    # defines dependencies)
                    nc.sync.dma_start(out=tw[:, :], in_=tw_d[:, :])
                    nc.sync.dma_start(out=ew[:, :], in_=ew_d[:, :])
                for j, k in enumerate(grp):
                    for q0, q1 in SLICES:
                        nc.tensor.matmul(ps_n[:, q0:q1],
                                         tw[:, k * BL:(k + 1) * BL],
                                         nk[:, j * R + q0:j * R + q1],
                                         start=(k == 0), stop=(k == KCH - 1))
                fk = f_pool.tile([P, 2 * R], bf16, tag="fk")
                nc.scalar.activation(out=fk[:, 0:g * R], in_=nk[:, 0:g * R],
                                     func=Act.Exp)
                for j, k in enumerate(grp):
                    for q0, q1 in SLICES:
                        nc.tensor.matmul(ps_s[:, q0:q1],
                                         ew[:, k * BL:(k + 1) * BL],
                                         fk[:, j * R + q0:j * R + q1],
                                         start=(k == 0), stop=(k == KCH - 1))

            gs_sb = out_pool.tile([BL, R], f32)
            gn_sb = out_pool.tile([BL, R], f32)
            nc.vector.tensor_copy(gn_sb[:, :], ps_n[:, :])
            nc.sync.dma_start(out=gn_d[:, :], in_=gn_sb[:, :])
            # split the gram_s drain across the two idle engines
            nc.scalar.copy(gs_sb[:, 0:800], ps_s[:, 0:800])
            nc.vector.tensor_copy(gs_sb[:, 800:R], ps_s[:, 800:R])
            nc.sync.dma_start(out=gs_d[:, :], in_=gs_sb[:, :])

    nc.compile()
    return nc


def _get_program():
    global _PROGRAM
    if _PROGRAM is None:
        _PROGRAM = _build_program()
    return _PROGRAM


def _run_on_device(in_maps, trace=False):
    from concourse.bass_utils import run_bass_kernel_spmd
    nc = _get_program()
    return run_bass_kernel_spmd(nc, in_maps, list(range(NCORES)), trace=trace)


def _weight_panels(mat_cb):
    """[C, BL] -> [P, KCH*BL] bf16: panel k holds rows 128k:128(k+1)."""
    out = np.zeros((CP, BL), np.float32)
    out[:C] = mat_cb
    return np.ascontiguousarray(
        out.reshape(KCH, P, BL).transpose(1, 0, 2).reshape(P, KCH * BL)
    ).astype(ml_dtypes.bfloat16)


def kernel(logit_var, pred, true, noise, _trace=False):
    logit_var = np.asarray(logit_var, dtype=np.float32)
    pred = np.asarray(pred, dtype=np.float32)
    true = np.asarray(true, dtype=np.float32)
    noise = np.asarray(noise, dtype=np.float32)

    std = np.sqrt(logit_var)                     # [B,1] f32
    bf = ml_dtypes.bfloat16

    in_maps = []
    for i in range(NCORES):
        b0 = i * BL
        bsl = slice(b0, b0 + BL)
        ns = noise[:, bsl, :] * std[None, bsl, :]     # [T,BL,C] f32
        nsT = np.zeros((CP, R), np.float32)
        nsT[:C] = ns.reshape(R, C).T
        in_maps.append({
            "noiset": nsT.astype(bf),
            "ew": _weight_panels(np.exp(pred[bsl]).T),
            "tw": _weight_panels(true[bsl].T),
        })

    out = _run_on_device(in_maps, trace=_trace)
    _LAST[0] = out
    results = out.results

    # Gather: gram column q = t*BL + b_local pairs with weight row b_local.
    sumexp = np.empty((T, B), np.float64)
    ndots = np.empty((T, B), np.float64)            # sum_c (std*noise)*true
    bidx = np.arange(R) % BL
    qidx = np.arange(R)
    for i in range(NCORES):
        r = results[i]
        gs = np.asarray(r["gs"], np.float64)        # [BL, R]
        gn = np.asarray(r["gn"], np.float64)
        b0 = i * BL
        sumexp[:, b0:b0 + BL] = gs[bidx, qidx].reshape(T, BL)
        ndots[:, b0:b0 + BL] = gn[bidx, qidx].reshape(T, BL)

    true64 = true.astype(np.float64)
    pred64 = pred.astype(np.float64)
    strue = true64.sum(axis=1)                      # [B]
    pdot = (pred64 * true64).sum(axis=1)            # [B]
    sexp_pred = np.exp(pred64).sum(axis=1)          # [B]

    lse = np.log(sumexp)                            # [T,B]
    dot = pdot[None, :] + ndots                     # [T,B]
    ce = (lse * strue[None, :] - dot) / 10.0        # [T,B]
    distorted = ce.mean(axis=1)                     # [T]
    undist = ((np.log(sexp_pred) * strue - pdot) / 10.0).mean()
    gce = distorted.mean()
    x = undist - distorted
    vloss = np.mean(-np.where(x > 0, x, np.expm1(x)))
    vd = np.mean(np.exp(logit_var[:, 0].astype(np.float64))) - 1.0

    return (np.float32(gce), np.float32(vloss), np.float32(undist),
            np.float32(vd))
